# revision 3
# baseline (speedup 1.0000x reference)
"""CapsuleLayer kernel for 8x TRN2 NeuronCores (Bass/Tile, SPMD).

Math (reference collapses because routing logits b stay zero):
  s[b,o,h,w]  = sum_ic conv2d(u[b,ic], W[ic], SAME) + sum_ic bias[ic]
              = conv2d(u[b] as 64ch, Wcat[256,64,5,5]) + bias_sum      (one conv)
  r(h,w)      = 1 / (8 * nvalid(h,w))              (input-independent constant)
  p           = r * s ; sq[oc] = sum_od p^2
  v           = p * sq/((1+sq)*sqrt(sq+1e-9))
  out[b,oc,od,h,w] = v

Sharding: 8 cores = (batch b in 0..4) x (OC half in 0..2). Each core runs a
64->128-channel 5x5 conv over one 128x128 image + squash, fully on-chip.

Conv as matmuls: 13 accumulating PE matmuls per 512-pixel tile with K=128.
Taps are packed two per matmul (2x64 in-channels in the 128 partitions):
  slots 0-9 : row pairs (dy,dy+1) for dy in {0,2} x kw in 0..4, via a
              second image copy shifted one ROW (partitions 64-127).
  slot 10   : single tap (4,4) (row-pair copy gives dy=5 = zero weights).
  slots 11-12: column pairs (4,kw)+(4,kw+1) for kw in {0,2}, via a third
              image copy shifted one COLUMN (partitions 64-127 of upadc).
fp16 operands (1 cyc/col, FWL weight loads, fp32 PSUM accumulate).

Squash: per-pixel cross-partition reduce/broadcast via PE matmuls with 0/1
masks, groups of 4 tiles. All matmuls the identical [K=128,M=128,N=512]
shape. Scalar chain uses only {square, ln, exp} (single ACT table set):
  G = r * exp(0.5*ln(sq+1e-9) - ln(1+sq));  v = s * G_broadcast  (f16 out)
Scheduling: software-pipelined emission (reduce lagged one conv tile,
phase2 backlog ~8 deep to bridge the final chain latency), PE warmup
matmuls during the input DMA, output tiles alternate both HWDGE rings.
"""

import numpy as np


def _ensure_path():
    try:
        import concourse.bass  # noqa: F401
    except ImportError:
        import sys

        for p in ("/opt/trn_rl_repo", "/root/.axon_site/_ro/trn_rl_repo"):
            if p not in sys.path:
                sys.path.insert(0, p)
        import concourse.bass  # noqa: F401


B, IC, CIN, H, W = 4, 4, 16, 128, 128
KS, OC, OD = 5, 8, 32
CC = IC * CIN            # 64 contraction channels
NOCH = 128               # out channels per core (4 capsules x 32 dims)
PADH, PADW = H + 5, W + 4   # 133 x 132 (extra pad row for the shifted copy)
NPIX = H * W
TPX = 512                # pixels per tile (4 rows)
NT = NPIX // TPX         # 32 tiles
GT = 4                   # tiles per squash group
GROUPS = [GT] * (NT // GT)   # 8 groups of 4
NGR = len(GROUPS)
NKT = 13                 # matmuls per conv tile (see header)

_BUILD_CACHE = {}


def _build_program():
    """Build the SPMD Bass program (same for every core)."""
    if "nc" in _BUILD_CACHE:
        return _BUILD_CACHE["nc"]
    _ensure_path()
    import concourse.bacc as bacc
    import concourse.mybir as mybir
    import concourse.tile as tile
    from concourse.tile import add_dep_helper

    f32 = mybir.dt.float32
    f16 = mybir.dt.float16
    AF = mybir.ActivationFunctionType
    OP = mybir.AluOpType

    # Square/Ln/Exp/Identity all live in the 'natural_log_exp_and_others' ACT
    # table set; restrict the picker so one table load covers the kernel.
    if not getattr(bacc, "_capsule_act_patch", False):
        _orig_tables = bacc.get_activation_tables

        def _one_set_tables(arch):
            t = _orig_tables(arch)
            keep = "natural_log_exp_and_others"
            if keep in t:
                t = {k: (v if k == keep else set()) for k, v in t.items()}
            return t

        bacc.get_activation_tables = _one_set_tables
        bacc._capsule_act_patch = True

    nc = bacc.Bacc("TRN2", target_bir_lowering=False, debug=False, num_devices=8)

    upad_d = nc.dram_tensor("upad", [128, PADH * PADW], f16, kind="ExternalInput").ap()
    upadc_d = nc.dram_tensor("upadc", [128, PADH * PADW], f16, kind="ExternalInput").ap()
    wt_d = nc.dram_tensor("wt", [128, NKT * NOCH], f16, kind="ExternalInput").ap()
    bias_d = nc.dram_tensor("bias", [128, 1], f32, kind="ExternalInput").ap()
    mred_d = nc.dram_tensor("mred", [128, GT * NOCH], f16, kind="ExternalInput").ap()
    sel_d = nc.dram_tensor("sel", [128, GT * NOCH], f16, kind="ExternalInput").ap()
    rr_d = nc.dram_tensor("rr", [4 * GT, 2 * NGR * TPX], f32, kind="ExternalInput").ap()
    out_d = nc.dram_tensor("out", [128, NPIX], f16, kind="ExternalOutput").ap()

    with tile.TileContext(nc) as tc:
        with (
            tc.tile_pool(name="const", bufs=1) as cpool,
            tc.tile_pool(name="sg", bufs=3) as sgpool,
            tc.tile_pool(name="sq", bufs=4) as sqpool,
            tc.tile_pool(name="chain", bufs=2) as chpool,
            tc.tile_pool(name="gv", bufs=3) as gvpool,
            tc.tile_pool(name="vout", bufs=3) as vpool,
            tc.tile_pool(name="cps", bufs=4, space="PSUM") as cps,
            tc.tile_pool(name="gps", bufs=2, space="PSUM") as gps,
            tc.tile_pool(name="bps", bufs=2, space="PSUM") as bps,
        ):
            # DMA order matters. Sync ring: wt (first conv needs it), then the
            # first upadc chunk (needed by tile 0's column-pair matmuls), then
            # masks, rr, rest of upadc. Scalar ring: the image in row chunks.
            wt_t = cpool.tile([128, NKT * NOCH], f16)
            nc.sync.dma_start(wt_t[:, 0 : 5 * NOCH], wt_d[:, 0 : 5 * NOCH])
            nc.sync.dma_start(wt_t[:, 5 * NOCH : 10 * NOCH], wt_d[:, 5 * NOCH : 10 * NOCH])
            nc.sync.dma_start(wt_t[:, 10 * NOCH :], wt_d[:, 10 * NOCH :])
            bias_t = cpool.tile([128, 1], f32)
            nc.sync.dma_start(bias_t[:], bias_d[:])

            upad_t = cpool.tile([128, PADH * PADW], f16)
            upad3 = upad_t[:].rearrange("p (y x) -> p y x", x=PADW)
            usrc3 = upad_d.rearrange("p (y x) -> p y x", x=PADW)
            upadc_t = cpool.tile([128, PADH * PADW], f16)
            upadc3 = upadc_t[:].rearrange("p (y x) -> p y x", x=PADW)
            ucsrc3 = upadc_d.rearrange("p (y x) -> p y x", x=PADW)
            row_chunks = [(0, 10), (10, 40), (40, 72), (72, 104), (104, PADH)]
            for r0, r1 in row_chunks:
                nc.scalar.dma_start(upad3[:, r0:r1, :], usrc3[:, r0:r1, :])
            # upadc rows 0-3 are never read (dy=4 taps start at padded row 4)
            nc.sync.dma_start(upadc3[:, 4:12, :], ucsrc3[:, 4:12, :])
            mred_t = cpool.tile([128, GT * NOCH], f16)
            nc.sync.dma_start(mred_t[:], mred_d[:])
            sel_t = cpool.tile([128, GT * NOCH], f16)
            nc.sync.dma_start(sel_t[:], sel_d[:])
            nc.sync.dma_start(upadc3[:, 12:40, :], ucsrc3[:, 12:40, :])
            rr_t = cpool.tile([128, 2 * NGR * TPX], f32)
            # rows >= 16 are never DMA'd but ARE read by the chain ops (junk
            # NaNs there would poison the bcast matmul: 0-weight x NaN = NaN).
            # Zero them once on the otherwise-idle GpSimd engine.
            nc.gpsimd.memset(rr_t[:], 0.0)
            nc.sync.dma_start(rr_t[0 : 4 * GT, :], rr_d[:])
            for r0, r1 in [(40, 72), (72, 104), (104, PADH)]:
                nc.sync.dma_start(upadc3[:, r0:r1, :], ucsrc3[:, r0:r1, :])
            eps_t = cpool.tile([128, 1], f32)
            nc.vector.memset(eps_t[:], 1e-9)

            # PE warmup: junk matmuls while the input DMAs land, so the HAM
            # clock gate is already at 8/8 (2.4 GHz) when real conv work
            # starts. Operands alias the const-0.0 tile (written in the Bass
            # preamble BEFORE the Tile entry barrier) bitcast to f16.
            warm_ps = bps.tile([128, TPX], f32, tag="bcast", name="warmps")
            c16 = nc.const_aps.aps[(f32, 0.0)].bitcast(f16)
            wrhs = c16[:, 0:1].to_broadcast((128, TPX))
            wlhs = c16[:, 0:1].to_broadcast((128, 128))
            NWARM = 14
            for k in range(NWARM):
                nc.tensor.matmul(
                    warm_ps[:],
                    wlhs,
                    wrhs,
                    start=(k == 0),
                    stop=(k == NWARM - 1),
                )

            first_tile = [GT * gi for gi in range(NGR)]

            s_tiles = {}
            g_tiles = {}
            gp_tiles = {}

            def emit_conv_tile(gi, j):
                t = first_tile[gi] + j
                y0 = 4 * t
                cp = cps.tile([128, TPX], f32, tag="convps")
                # (view, slot) pairs; upadc-dependent slots last for DMA slack
                rhss = []
                for dyp in range(2):
                    for kw in range(KS):
                        rhss.append(upad3[:, y0 + 2 * dyp : y0 + 2 * dyp + 4, kw : kw + W])
                rhss.append(upad3[:, y0 + 4 : y0 + 8, 4 : 4 + W])      # (4,4)
                rhss.append(upadc3[:, y0 + 4 : y0 + 8, 0:W])           # (4,0)+(4,1)
                rhss.append(upadc3[:, y0 + 4 : y0 + 8, 2 : 2 + W])     # (4,2)+(4,3)
                last_mm = None
                for ti, rhs in enumerate(rhss):
                    last_mm = nc.tensor.matmul(
                        cp[:],
                        wt_t[:, ti * NOCH : (ti + 1) * NOCH],
                        rhs,
                        start=(ti == 0),
                        stop=(ti == NKT - 1),
                    )
                # Square(cp + bias) straight from PSUM (fused bias)
                sq = sqpool.tile([128, TPX], f16, tag="sqt")
                nc.scalar.activation(sq[:], cp[:], AF.Square, bias=bias_t[:, 0:1])
                s_sl = s_tiles[gi][:, j * TPX : (j + 1) * TPX]
                nc.scalar.add(s_sl, cp[:], bias_t[:, 0:1])
                return sq, last_mm

            def emit_red(gi, j, sq, anchor):
                gsz = GROUPS[gi]
                red_mm = nc.tensor.matmul(
                    gp_tiles[gi][:],
                    mred_t[:, j * NOCH : (j + 1) * NOCH],
                    sq[:],
                    start=(j == 0),
                    stop=(j == gsz - 1),
                )
                if anchor is not None:
                    # keep the reduce AFTER the just-emitted conv tile in the
                    # PE stream so its ACT square input is long done
                    add_dep_helper(
                        red_mm.ins, anchor.ins, sync=True, reason="lag red"
                    )
                return red_mm

            def emit_chain(gi):
                gp = gp_tiles[gi]
                r4 = rr_t[:, gi * TPX : (gi + 1) * TPX]
                r1 = rr_t[:, (NGR + gi) * TPX : (NGR + gi + 1) * TPX]
                sqv = chpool.tile([128, TPX], f32, tag="sqv")
                nc.vector.tensor_mul(sqv[:], gp[:], r4)
                ln_a = chpool.tile([128, TPX], f32, tag="ln_a")
                nc.scalar.activation(ln_a[:], sqv[:], AF.Ln, bias=eps_t[:, 0:1])
                ln_b = chpool.tile([128, TPX], f32, tag="ln_b")
                nc.scalar.activation(ln_b[:], sqv[:], AF.Ln, bias=1.0)
                dd = chpool.tile([128, TPX], f32, tag="dd")
                nc.vector.scalar_tensor_tensor(
                    dd[:], ln_a[:], 0.5, ln_b[:], OP.mult, OP.subtract
                )
                ee = chpool.tile([128, TPX], f32, tag="ee")
                nc.scalar.activation(ee[:], dd[:], AF.Exp)
                gt_ = gvpool.tile([128, TPX], f16, tag="g32")
                nc.vector.tensor_mul(gt_[:], ee[:], r1)
                g_tiles[gi] = gt_

            def emit_phase2(gi, j, anchor=None):
                t = first_tile[gi] + j
                gb = bps.tile([128, TPX], f32, tag="bcast")
                bc_mm = nc.tensor.matmul(
                    gb[:],
                    sel_t[:, j * NOCH : (j + 1) * NOCH],
                    g_tiles[gi][:],
                    start=True,
                    stop=True,
                )
                if anchor is not None:
                    add_dep_helper(
                        bc_mm.ins, anchor.ins, sync=True, reason="lag bcast"
                    )
                v = vpool.tile([128, TPX], f16, tag="vout")
                s_sl = s_tiles[gi][:, j * TPX : (j + 1) * TPX]
                nc.vector.tensor_mul(v[:], s_sl, gb[:])
                eng = nc.sync if (t % 2 == 0) else nc.scalar
                eng.dma_start(out_d[:, t * TPX : (t + 1) * TPX], v[:])

            # Software-pipelined emission: the reduce for a tile is emitted one
            # conv-tile later (covers the ACT add+square latency), the chain as
            # soon as the group's last reduce is out, and phase2 work of group
            # g drains while later groups' convs keep the PE busy. A backlog of
            # ~HOLD phase2 items is kept to bridge the final chain's latency.
            from collections import deque

            HOLD = 8
            pend_red = deque()      # (gi, j, sq_tile)
            pend_p2 = deque()       # (gi, j)
            tiles_left = NT
            for gi, gsz in enumerate(GROUPS):
                s_tiles[gi] = sgpool.tile(
                    [128, gsz * TPX], f32, tag="sgroup", name=f"sgroup{gi}"
                )
                gp_tiles[gi] = gps.tile(
                    [128, TPX], f32, tag="redps", name=f"redps{gi}"
                )
                for j in range(gsz):
                    sq_j, last_mm = emit_conv_tile(gi, j)
                    tiles_left -= 1
                    if pend_red:
                        rgi, rj, rsq = pend_red.popleft()
                        emit_red(rgi, rj, rsq, last_mm)
                        if rj == GROUPS[rgi] - 1:
                            emit_chain(rgi)
                            pend_p2.extend((rgi, k) for k in range(GROUPS[rgi]))
                    pend_red.append((gi, j, sq_j))
                    excess = len(pend_p2) - HOLD
                    if excess > 0:
                        npop = -(-excess // max(1, tiles_left))
                        for _ in range(min(npop, len(pend_p2))):
                            emit_phase2(*pend_p2.popleft(), anchor=last_mm)
            # drain
            prev_mm = last_mm
            while pend_red:
                rgi, rj, rsq = pend_red.popleft()
                prev_mm = emit_red(rgi, rj, rsq, prev_mm)
                if rj == GROUPS[rgi] - 1:
                    emit_chain(rgi)
                    pend_p2.extend((rgi, k) for k in range(GROUPS[rgi]))
            while pend_p2:
                emit_phase2(*pend_p2.popleft(), anchor=prev_mm)

    nc.compile()
    _BUILD_CACHE["nc"] = nc
    return nc


def _host_prep(u, Wf, bias):
    """Per-core input arrays. u [4,4,16,128,128], Wf [4,256,16,5,5], bias [4,256]."""
    u = np.ascontiguousarray(u, dtype=np.float32)
    Wf = np.ascontiguousarray(Wf, dtype=np.float32)
    bias = np.ascontiguousarray(bias, dtype=np.float32)

    # r(h,w) = 1/(8*nvalid); nvalid = clipped 5x5 window size
    nv = np.minimum(np.arange(H) + 2, H - 1) - np.maximum(np.arange(H) - 2, 0) + 1
    nvalid = np.outer(nv, nv).astype(np.float64)
    r = (1.0 / (8.0 * nvalid)).astype(np.float32)          # [H, W]

    # RR[p, gi*TPX + n]: row p -> tile j = p//4 of group gi (16 rows used)
    rr = np.zeros((4 * GT, 2 * NGR * TPX), np.float32)
    rflat = r.reshape(H * W)
    for gi in range(NGR):
        for p in range(4 * GT):
            t = GT * gi + p // 4
            px = rflat[t * TPX : (t + 1) * TPX]
            rr[p, gi * TPX : (gi + 1) * TPX] = px * px
            rr[p, (NGR + gi) * TPX : (NGR + gi + 1) * TPX] = px

    # reduce masks: MRED[p, j*128+m] = 1 if m == 4*j + p//32
    # broadcast sel: SEL[p, j*128+m] = 1 if p == 4*j + m//32
    mred = np.zeros((128, GT * NOCH), np.float16)
    for j in range(GT):
        for p in range(128):
            mred[p, j * NOCH + 4 * j + p // 32] = 1.0
    sel = np.zeros((128, GT * NOCH), np.float16)
    for j in range(GT):
        for m in range(NOCH):
            sel[4 * j + m // 32, j * NOCH + m] = 1.0

    bias_sum = bias.sum(axis=0)                            # [256]

    # weights WT[p, slot*128 + o]; p = h64*64 + ic*16 + cid
    #  slots 0-9 : (dy = 2*dyp + h64, kw), dyp in {0,1}, kw 0..4
    #  slot 10   : h64=0 -> (4,4); h64=1 -> zero
    #  slot 11   : h64=0 -> (4,0); h64=1 -> (4,1)
    #  slot 12   : h64=0 -> (4,2); h64=1 -> (4,3)
    wts = []
    for half in range(2):
        wt = np.zeros((128, NKT * NOCH), np.float16)
        Wh = Wf[:, half * NOCH : (half + 1) * NOCH]        # [4, 128, 16, 5, 5]

        def blk(dy, kw):
            # [4,128,16] -> [4,16,128] -> [64,128]
            return Wh[:, :, :, dy, kw].transpose(0, 2, 1).reshape(64, NOCH)

        for dyp in range(2):
            for kw in range(KS):
                ti = dyp * 5 + kw
                for h64 in range(2):
                    wt[h64 * 64 : (h64 + 1) * 64, ti * NOCH : (ti + 1) * NOCH] = blk(
                        2 * dyp + h64, kw
                    )
        wt[0:64, 10 * NOCH : 11 * NOCH] = blk(4, 4)
        wt[0:64, 11 * NOCH : 12 * NOCH] = blk(4, 0)
        wt[64:128, 11 * NOCH : 12 * NOCH] = blk(4, 1)
        wt[0:64, 12 * NOCH : 13 * NOCH] = blk(4, 2)
        wt[64:128, 12 * NOCH : 13 * NOCH] = blk(4, 3)
        wts.append(wt)

    # padded images per batch: upad rows 64-127 = row(+1)-shifted copy,
    # upadc rows 64-127 = column(+1)-shifted copy
    upads, upadcs = [], []
    for b in range(B):
        pad = np.zeros((CC, PADH, PADW), np.float16)
        pad[:, 2 : 2 + H, 2 : 2 + W] = u[b].reshape(CC, H, W)
        up = np.empty((128, PADH * PADW), np.float16)
        up[0:64] = pad.reshape(CC, -1)
        sh = np.zeros_like(pad)
        sh[:, 0 : PADH - 1] = pad[:, 1:PADH]
        up[64:128] = sh.reshape(CC, -1)
        upads.append(up)
        upc = np.empty((128, PADH * PADW), np.float16)
        upc[0:64] = pad.reshape(CC, -1)
        shc = np.zeros_like(pad)
        shc[:, :, 0 : PADW - 1] = pad[:, :, 1:PADW]
        upc[64:128] = shc.reshape(CC, -1)
        upadcs.append(upc)

    in_maps = []
    for c in range(8):
        b, half = c // 2, c % 2
        in_maps.append(
            {
                "upad": upads[b],
                "upadc": upadcs[b],
                "wt": wts[half],
                "bias": bias_sum[half * NOCH : (half + 1) * NOCH]
                .reshape(128, 1)
                .copy(),
                "mred": mred,
                "sel": sel,
                "rr": rr,
            }
        )
    return in_maps


def _gather(results):
    out = np.empty((B, OC, OD, H, W), np.float32)
    for c in range(8):
        b, half = c // 2, c % 2
        o = results[c]["out"]                              # [128, NPIX] f16
        out[b, half * 4 : (half + 1) * 4] = o.reshape(4, OD, H, W).astype(np.float32)
    return out


def run(u, W, bias, trace=False):
    _ensure_path()
    from concourse.bass_utils import run_bass_kernel_spmd

    nc = _build_program()
    in_maps = _host_prep(u, W, bias)
    res = run_bass_kernel_spmd(nc, in_maps, list(range(8)), trace=trace)
    return _gather(res.results), res


def kernel(u, W, bias):
    out, _ = run(u, W, bias, trace=False)
    return out
